# revision 36
# baseline (speedup 1.0000x reference)
"""Trainium2 Bass kernel for nn_MultiHeadAttention_88923002896848.

MHA with KV-cache concat: out = MHA(query; [cache;key_in]; [cache;value_in]).
Shapes: B=128, T1=188, LC=70, T2=258, F=512, H=8, DK=64. fp32 I/O.

Strategy (8 NeuronCores, data-parallel over batch, 16 batches/core):
  - Host packs all inputs into the exact on-chip layouts (partition-major),
    so every input DMA is one contiguous run per partition.
  - All matmuls fp16. Attention transposed (S^T = k-chunks x q) so exp output
    E^T feeds PV with zero on-chip transposes; exp is max-free (logits ~N(0,1))
    with a constant -3 shift; softmax denominator comes free from a ones-column
    appended to time-major V (row 64 of the PV psum).
  - Normalization runs entirely off the PE: Vector reciprocal of the
    denominator row, GpSimd partition_broadcast across the 64 head dims,
    Vector multiply into the (feature-major) context buffer.
  - Output projection is pair-folded feature-major (N=376, M=128 always);
    the kernel emits outT [b, F, T1] fp16 and the host transposes/upcasts.
  - Software pipelining across batch pairs: while pair p's attention runs,
    the PE also executes Q/K/V projections of pair p+1 and the output
    projection of pair p-1 as filler groups slotted between attention
    groups, keeping the PE dense so the HAM clock gate stays at 2.4 GHz.
"""

import numpy as np

NCORES = 8
B, T1, LC, F, H = 128, 188, 70, 512, 8
DK = F // H            # 64
T2 = LC + T1           # 258
P = 128
KO = F // P            # 4 fin/fout tiles of 128
NB = B // NCORES       # 16 batches per core
NPAIR = NB // 2
SCALE = 1.0 / np.sqrt(DK)
EXP_SHIFT = -3.0       # exp(scale*s + shift); cancels in the softmax ratio

# T2 chunks aligned to the cache/key seam: (size, (source, t0, t1))
T2_CHUNKS = [(LC, ("cache", 0, LC)), (128, ("key", 0, 128)), (T1 - 128, ("key", 128, T1))]

_BUILT = None


def _build():
    import concourse.bacc as bacc
    import concourse.mybir as mybir
    import concourse.tile as tile
    from concourse.bass import ts
    from contextlib import ExitStack

    dt = mybir.dt
    f32, f16 = dt.float32, dt.float16
    AF = mybir.ActivationFunctionType

    nc = bacc.Bacc(trn_type="TRN2")

    qT = nc.dram_tensor("qT", [NPAIR, P, KO, 2, T1], f16, kind="ExternalInput")
    keyT = nc.dram_tensor("keyT", [NPAIR, P, KO, 2, T1], f16, kind="ExternalInput")
    valT = nc.dram_tensor("valT", [NPAIR, P, KO, 2, T1], f16, kind="ExternalInput")
    cachT = nc.dram_tensor("cachT", [P, KO, NB, LC], f16, kind="ExternalInput")
    wq_d = nc.dram_tensor("wq", [P, KO, F], f16, kind="ExternalInput")
    wk_d = nc.dram_tensor("wk", [P, KO, F], f16, kind="ExternalInput")
    wv_d = nc.dram_tensor("wv", [P, KO, F], f16, kind="ExternalInput")
    wo_d = nc.dram_tensor("wo", [P, KO, F], f16, kind="ExternalInput")
    out_d = nc.dram_tensor("out", [NB, F, T1], f16, kind="ExternalOutput")

    with tile.TileContext(nc) as tc, ExitStack() as ctx:
        consts = ctx.enter_context(tc.tile_pool(name="consts", bufs=1))
        iobuf = ctx.enter_context(tc.tile_pool(name="iobuf", bufs=2))
        act16 = ctx.enter_context(tc.tile_pool(name="act16", bufs=2))
        small = ctx.enter_context(tc.tile_pool(name="small", bufs=2))
        pproj = ctx.enter_context(tc.tile_pool(name="pproj", bufs=2, space="PSUM"))
        # pss tiles are 2 banks each ([P, 2, 2, 256] f32); 2 bufs = 4 banks
        pscore = ctx.enter_context(tc.tile_pool(name="pscore", bufs=2, space="PSUM"))
        pctx = ctx.enter_context(tc.tile_pool(name="pctx", bufs=2, space="PSUM"))

        # ---- constants ----
        # DMA order: wk + cache first so the KTc preamble matmuls can start
        # as early as possible, overlapping the remaining input streams
        w_sb = {}
        for nm, drt in (("wk", wk_d), ("wq", wq_d), ("wv", wv_d), ("wo", wo_d)):
            wt = consts.tile([P, KO, F], f16, name=f"{nm}_sb", tag=f"{nm}_sb")
            w_sb[nm] = wt
        cache_all = consts.tile([P, KO, NB, LC], f16, name="cache_all")
        nc.sync.dma_start(
            w_sb["wk"][:, :2].rearrange("p o f -> p (o f)"),
            wk_d[:, :2].rearrange("p o f -> p (o f)"),
        )
        nc.sync.dma_start(
            w_sb["wk"][:, 2:].rearrange("p o f -> p (o f)"),
            wk_d[:, 2:].rearrange("p o f -> p (o f)"),
        )
        # cache lands in two halves so the first KTc chunk (batches 0-5)
        # can start while the second half streams
        nc.sync.dma_start(cache_all[:, :, :6], cachT[:, :, :6])
        nc.sync.dma_start(cache_all[:, :, 6:], cachT[:, :, 6:])
        nc.sync.dma_start(
            w_sb["wq"].rearrange("p o f -> p (o f)"),
            wq_d.rearrange("p o f -> p (o f)"),
        )
        biasm3 = consts.tile([P, 1], f32, name="biasm3")
        nc.vector.memset(biasm3[:], EXP_SHIFT)

        # ---- HAM keep-warm: dependency-free matmuls on a zero tile ----
        # The PE clock gate defaults to 4/8 (1.2 GHz) and only reaches 2.4 GHz
        # after ~3.4us of sustained activity; it re-throttles after ~3.4us
        # idle.  Dummy matmuls spin the array while real work is blocked on
        # DMA (head) or the normalization chain (tail).
        dumw = consts.tile([P, F], f16, name="dumw")
        nc.vector.memset(dumw.rearrange("p f -> p (f)"), 0.0)

        def dummy_mm(n=1):
            for _ in range(n):
                pdm = pproj.tile([P, F], f32, tag="proj", name="pdm")
                nc.tensor.matmul(
                    pdm[:], dumw[:, 0:P], dumw[:], start=True, stop=True
                )

        # spin the PE during the initial weight/cache DMA wait so the KTc
        # preamble starts already at 2.4 GHz; 12 cold matmuls (~5us) both
        # cover the HAM activity window and bridge to the first real matmul
        dummy_mm(12)

        # ---- zero the K-pad rows of the attention operand tiles ----
        # PV matmuls run with a full K=128 contraction (pad rows contribute
        # exact zeros) so the weight-load path sees 128-row stationaries and
        # overlaps LDWEIGHTS with the running matmul, like the projections.
        # Both rotation buffers of each tag are zeroed once; steady-state
        # writers only ever touch rows [:tcn], so the pads stay zero.
        for _ in range(2):
            for ci, (tcn, _) in enumerate(T2_CHUNKS):
                if tcn < P:
                    Ez = act16.tile(
                        [P, KO, 2, 2, T1], f16, tag=f"E{ci}", name="Ez", bufs=2
                    )
                    nc.vector.memset(
                        Ez.rearrange("p a b c t -> p (a b c t)"), 0.0
                    )
                    for lb in range(2):
                        vz = act16.tile(
                            [P, H, DK + 1], f16, tag=f"vt{lb}{ci}", name="vz", bufs=2
                        )
                        nc.vector.memset(
                            vz.rearrange("p h d -> p (h d)"), 0.0
                        )

        # ---- K projection of all cache frames (feature-major, fp16) ----
        # chunk-outer so chunk 0 (cache batches 0-5) only waits for the
        # first half of the cache DMA
        KTc = consts.tile([P, KO, NB, LC], f16, name="KTc")
        NTOT = NB * LC  # 1120
        cch = [(0, 374), (374, 374), (748, NTOT - 748)]
        for c0, cn in cch:
            for fo in range(KO):
                pkc = pproj.tile([P, F], f32, tag="proj", name="pkc")
                for k in range(KO):
                    nc.tensor.matmul(
                        pkc[:, :cn],
                        w_sb["wk"][:, k, ts(fo, P)],
                        cache_all[:, k].rearrange("p b t -> p (b t)")[:, c0 : c0 + cn],
                        start=(k == 0),
                        stop=(k == KO - 1),
                    )
                nc.scalar.copy(
                    KTc[:, fo].rearrange("p b t -> p (b t)")[:, c0 : c0 + cn],
                    pkc[:, :cn],
                )

        # ================= pipelined stages =================

        def dma_pair(pr):
            qp = iobuf.tile([P, KO, 2, T1], f16, tag="qp", name="qp")
            kp = iobuf.tile([P, KO, 2, T1], f16, tag="kp", name="kp")
            vp = iobuf.tile([P, KO, 2, T1], f16, tag="vp", name="vp")
            nc.sync.dma_start(
                qp.rearrange("p o b t -> p (o b t)"),
                qT[pr].rearrange("p o b t -> p (o b t)"),
            )
            nc.sync.dma_start(
                kp.rearrange("p o b t -> p (o b t)"),
                keyT[pr].rearrange("p o b t -> p (o b t)"),
            )
            nc.sync.dma_start(
                vp.rearrange("p o b t -> p (o b t)"),
                valT[pr].rearrange("p o b t -> p (o b t)"),
            )
            return qp, kp, vp

        def qk_proj_group(io, w, dst, fo):
            # one fo block of the pair-folded Q or K projection (N=376)
            pq = pproj.tile([P, F], f32, tag="proj", name="pq")
            for k in range(KO):
                nc.tensor.matmul(
                    pq[:, : 2 * T1],
                    w_sb[w][:, k, ts(fo, P)],
                    io[:, k].rearrange("p b t -> p (b t)"),
                    start=(k == 0),
                    stop=(k == KO - 1),
                )
            if w == "wq":
                nc.scalar.copy(dst[:, fo].rearrange("p b t -> p (b t)"), pq[:, : 2 * T1])
            else:
                nc.vector.tensor_copy(
                    dst[:, fo].rearrange("p b t -> p (b t)"), pq[:, : 2 * T1]
                )

        def v_proj_group(vp, pr, lb, ci, vts):
            # one T2 chunk of batch (pr, lb)'s V projection, time-major + ones col
            tcn, (src, s0, s1) = T2_CHUNKS[ci]
            b = 2 * pr + lb
            pv = pproj.tile([P, F], f32, tag="proj", name="pv")[:tcn]
            for k in range(KO):
                if src == "cache":
                    lhsT = cache_all[:, k, b, :]
                else:
                    lhsT = vp[:, k, lb, s0:s1]
                nc.tensor.matmul(
                    pv[:, :],
                    lhsT,
                    w_sb["wv"][:, k],
                    start=(k == 0),
                    stop=(k == KO - 1),
                )
            vt = vts[lb][ci]
            nc.vector.tensor_copy(vt[:tcn, :, 0:DK], pv.rearrange("t (h d) -> t h d", d=DK))
            nc.gpsimd.memset(vt[:tcn, :, DK : DK + 1], 1.0)

        obp_live = {}

        def o_proj_group(ctxs, pr, fo, lbs=(0, 1), eager=False):
            # one fo block of the pair-folded output projection (N=376),
            # feature-major output; lbs selects batches (epilogue splits them).
            # Drains accumulate in a per-(pair, lbs) staging tile; the DRAM
            # store is one batched DMA per batch after the last fo block.
            nlb = len(lbs)
            po = pproj.tile([P, F], f32, tag="proj", name="po")
            for k in range(KO):
                if nlb == 2:
                    rhs = ctxs[:, k].rearrange("p b t -> p (b t)")
                else:
                    rhs = ctxs[:, k, lbs[0], :]
                nc.tensor.matmul(
                    po[:, : nlb * T1],
                    w_sb["wo"][:, k, ts(fo, P)],
                    rhs,
                    start=(k == 0),
                    stop=(k == KO - 1),
                )
            if eager:
                # epilogue: store each fo block as soon as it drains (shorter
                # critical path than the batched whole-row store), and drain
                # on DVE since the scalar engine is saturated with exp here
                ob = small.tile([P, 2, T1], f16, tag="obe", name="obe", bufs=3)
                nc.vector.tensor_copy(
                    ob[:, :nlb].rearrange("p b t -> p (b t)"), po[:, : nlb * T1]
                )
                for i, lb in enumerate(lbs):
                    nc.sync.dma_start(out_d[2 * pr + lb, ts(fo, P), :], ob[:, i, :])
                return
            key = (pr, lbs)
            if key not in obp_live:
                obp_live[key] = small.tile(
                    [P, KO, 2, T1], f16, tag="obp", name="obp", bufs=2
                )
            obp = obp_live[key]
            nc.scalar.copy(
                obp[:, fo, :nlb].rearrange("p b t -> p (b t)"), po[:, : nlb * T1]
            )
            if fo == KO - 1:
                for i, lb in enumerate(lbs):
                    b = 2 * pr + lb
                    nc.sync.dma_start(
                        out_d[b].rearrange("(o p) t -> p o t", p=P),
                        obp[:, :, i, :],
                    )
                del obp_live[key]

        # ---- prologue: pair 0 inputs + projections ----
        qp0, kp0, vp0 = dma_pair(0)
        for nm, drt in (("wv", wv_d), ("wo", wo_d)):
            nc.sync.dma_start(
                w_sb[nm].rearrange("p o f -> p (o f)"),
                drt.rearrange("p o f -> p (o f)"),
            )
        q16 = act16.tile([P, KO, 2, T1], f16, tag="q16", name="q16")
        k16 = act16.tile([P, KO, 2, T1], f16, tag="k16", name="k16")
        for fo in range(KO):
            qk_proj_group(qp0, "wq", q16, fo)
        for fo in range(KO):
            qk_proj_group(kp0, "wk", k16, fo)
        vts = [
            [
                act16.tile([P, H, DK + 1], f16, tag=f"vt{lb}{ci}", name=f"vt{lb}{ci}", bufs=2)
                for ci, (tcn, _) in enumerate(T2_CHUNKS)
            ]
            for lb in range(2)
        ]
        for lb in range(2):
            for ci in range(len(T2_CHUNKS)):
                v_proj_group(vp0, 0, lb, ci, vts)

        prev = None  # (pr, ctxs) of previous pair, for the deferred O projection
        vts_deferred = None  # (vp, vts) of the last pair, projected in-pair

        for pr in range(NPAIR):
            # queue next pair's DMA + build its filler group list.  The last
            # pair's V projection is deferred into the last pair itself (VP
            # fillers first there) so the tail keeps the PE dense.
            fillers = []
            if pr == NPAIR - 1 and vts_deferred is not None:
                vpn_d, vtsn_d = vts_deferred
                for lb in range(2):
                    for ci in range(len(T2_CHUNKS)):
                        fillers.append(
                            lambda lb=lb, ci=ci: v_proj_group(vpn_d, pr, lb, ci, vtsn_d)
                        )
                vts = vtsn_d
            if pr + 1 < NPAIR:
                qpn, kpn, vpn = dma_pair(pr + 1)
                q16n = act16.tile([P, KO, 2, T1], f16, tag="q16", name="q16n")
                k16n = act16.tile([P, KO, 2, T1], f16, tag="k16", name="k16n")
                vtsn = [
                    [
                        act16.tile(
                            [P, H, DK + 1], f16, tag=f"vt{lb}{ci}", name=f"vtn{lb}{ci}", bufs=2
                        )
                        for ci, (tcn, _) in enumerate(T2_CHUNKS)
                    ]
                    for lb in range(2)
                ]
                for fo in range(KO):
                    fillers.append(lambda fo=fo: qk_proj_group(qpn, "wq", q16n, fo))
                for fo in range(KO):
                    fillers.append(lambda fo=fo: qk_proj_group(kpn, "wk", k16n, fo))
                if pr + 1 == NPAIR - 1:
                    vts_deferred = (vpn, vtsn)
                    vtsn = None
                else:
                    for lb in range(2):
                        for ci in range(len(T2_CHUNKS)):
                            fillers.append(
                                lambda lb=lb, ci=ci: v_proj_group(vpn, pr + 1, lb, ci, vtsn)
                            )
            else:
                q16n = k16n = vtsn = None
            if prev is not None:
                ppr, pctxs = prev
                for fo in range(KO):
                    fillers.append(lambda fo=fo: o_proj_group(pctxs, ppr, fo))
            fillers.reverse()  # pop() from the front

            def filler(n=1):
                for _ in range(n):
                    if fillers:
                        fillers.pop()()
                    elif pr == NPAIR - 1:
                        # no real work left: keep the PE clock gate warm
                        dummy_mm()

            q16c, k16c, vtsc = q16, k16, vts

            # E tiles for the pair: [tcn, fo, j, lb, t] fp16 (h = 2*fo + j)
            E = [
                act16.tile([P, KO, 2, 2, T1], f16, tag=f"E{ci}", name=f"E{ci}", bufs=2)
                for ci, (tcn, _) in enumerate(T2_CHUNKS)
            ]
            cu = [
                small.tile([DK + 1, H, T1], f16, tag=f"cu{lb}", name=f"cu{lb}", bufs=2)
                for lb in range(2)
            ]
            ctxs = small.tile([P, KO, 2, T1], f16, tag="ctxs", name="ctxs", bufs=2)

            def scores_fo(fo):
                # S^T + exp for head pair fo, both batches, all chunks.
                # One 2-bank psum tile per chunk holds all four (j, lb) slots;
                # emission alternates j so consecutive matmuls land on
                # disjoint PE row halves (rows 0-63 vs 64-127) and run
                # concurrently via tile_position row groups.  lb=0 starts
                # (whole-bank clear), lb=1 lands on cleared has_written bits.
                for ci, (tcn, (src, s0, s1)) in enumerate(T2_CHUNKS):
                    pss = pscore.tile([P, 2, 2, 256], f32, tag="pss", name="pss")[:tcn]
                    for lb in range(2):
                        for j in range(2):
                            if src == "cache":
                                lhsT = KTc[ts(j, DK), fo, 2 * pr + lb, :]
                            else:
                                lhsT = k16c[ts(j, DK), fo, lb, s0:s1]
                            nc.tensor.matmul(
                                pss[:, j, lb, :T1],
                                lhsT,
                                q16c[ts(j, DK), fo, lb, :],
                                start=(lb == 0),
                                stop=(lb == 1),
                                skip_group_check=True,
                            )
                    # one wide activation covers both heads and both batches
                    nc.scalar.activation(
                        E[ci][:tcn, fo, :, :, :],
                        pss[:, :, :, :T1],
                        AF.Exp,
                        bias=biasm3[:tcn, :],
                        scale=SCALE,
                    )

            def pv_fo(fo, lbs=(0, 1), drain_scalar=False):
                # PV with fused denominator row; drain unnormalized to cu (fp16)
                for lb in lbs:
                    pc = pctx.tile([DK + 1, 2, 256], f32, tag="pc", name="pc")
                    for j in range(2):
                        h = 2 * fo + j
                        for ci, (tcn, _) in enumerate(T2_CHUNKS):
                            # both heads share one accumulation group (j=1's
                            # first write overwrites on cleared bits) to avoid
                            # a second bank-clear and mid-group drain tail.
                            # K is padded to the full 128 rows (pads are zero)
                            # so the weight load takes the fast 128-row path.
                            nc.tensor.matmul(
                                pc[:, j, :T1],
                                vtsc[lb][ci][:, h, :],
                                E[ci][:, fo, j, lb, :],
                                start=(j == 0 and ci == 0),
                                stop=(j == 1 and ci == len(T2_CHUNKS) - 1),
                            )
                    if drain_scalar:
                        # epilogue: scalar is free after the last exp, and
                        # keeping these drains off the DVE lets the norm
                        # chains start the moment their cu rows complete
                        nc.scalar.copy(
                            cu[lb][:, 2 * fo : 2 * fo + 2, :], pc[:, :, :T1]
                        )
                    else:
                        nc.vector.tensor_copy(
                            cu[lb][:, 2 * fo : 2 * fo + 2, :], pc[:, :, :T1]
                        )

            # ---- attention sequence with interleaved filler groups ----
            if pr < NPAIR - 1:
                scores_fo(0)
                filler(2)
                scores_fo(1)
                filler(2)
                pv_fo(0)
                filler(2)
                scores_fo(2)
                filler(2)
                pv_fo(1)
                filler(2)
                scores_fo(3)
                filler(2)
                pv_fo(2)
                filler(2)
                pv_fo(3)
                while fillers:
                    filler(1)

            # ---- normalization (no PE): recip + partition broadcast + mul ----
            # The two batches' denominator rows (2x1504 fp16 on one partition)
            # are DMA-packed onto 94 partitions so the f32 cast + reciprocal +
            # f16 cast run at FD=32 instead of FD=1504 per instruction, then
            # DMA-unpacked back to single-partition layout for the gpsimd
            # broadcast.  Saves ~7us/pair of single-lane DVE work.
            def norm(lbs=(0, 1)):
                nlb = len(lbs)
                dpk = small.tile([94, 32], f16, tag="dpk", name="dpk", bufs=2)
                for i, lb in enumerate(lbs):
                    nc.sync.dma_start(
                        dpk[47 * i : 47 * (i + 1), :],
                        cu[lb][DK : DK + 1, :, :].rearrange("p h t -> p (h t)"),
                    )
                dpk32 = small.tile([94, 32], f32, tag="dpk32", name="dpk32", bufs=2)
                nc.vector.tensor_copy(dpk32[: 47 * nlb], dpk[: 47 * nlb])
                rpk32 = small.tile([94, 32], f32, tag="rpk32", name="rpk32", bufs=2)
                nc.vector.reciprocal_approx_fast(
                    out=rpk32[: 47 * nlb], in_=dpk32[: 47 * nlb]
                )
                rpk = small.tile([94, 32], f16, tag="rpk", name="rpk", bufs=2)
                nc.vector.tensor_copy(rpk[: 47 * nlb], rpk32[: 47 * nlb])
                rjs = {}
                for i, lb in enumerate(lbs):
                    rj = small.tile([1, H, T1], f16, tag=f"rj{lb}", name=f"rj{lb}", bufs=2)
                    nc.sync.dma_start(
                        rj.rearrange("p h t -> p (h t)"),
                        rpk[47 * i : 47 * (i + 1), :],
                    )
                    rjs[lb] = rj
                for lb in lbs:
                    for j in range(2):
                        rb = small.tile(
                            [DK, KO, T1], f16, tag=f"rb{j}", name=f"rb{j}", bufs=2
                        )
                        nc.gpsimd.partition_broadcast(
                            rb[:],
                            rjs[lb].rearrange("p (f j) t -> p j f t", j=2)[:, j],
                        )
                        nc.vector.tensor_mul(
                            ctxs[ts(j, DK), :, lb, :],
                            cu[lb][0:DK, :, :].rearrange("p (f j) t -> p j f t", j=2)[
                                :, j
                            ],
                            rb[:],
                        )

            if pr == NPAIR - 1:
                # final pair: drain batch 0's attention first so its
                # normalization and output projection overlap batch 1's PV,
                # keeping the PE busy through the pipeline tail
                scores_fo(0)
                filler(3)
                scores_fo(1)
                filler(3)
                pv_fo(0, lbs=(0,), drain_scalar=True)
                scores_fo(2)
                filler(2)
                pv_fo(1, lbs=(0,), drain_scalar=True)
                scores_fo(3)
                filler(2)
                pv_fo(2, lbs=(0,), drain_scalar=True)
                filler(1)
                pv_fo(3, lbs=(0,), drain_scalar=True)
                while fillers:
                    filler(1)
                # batch 0's norm chain (DMA/DVE/gpsimd) runs under batch 1's
                # PV; cu drains go to scalar so the DVE is free for the norms
                norm(lbs=(0,))
                pv_fo(0, lbs=(1,), drain_scalar=True)
                pv_fo(1, lbs=(1,), drain_scalar=True)
                pv_fo(2, lbs=(1,), drain_scalar=True)
                pv_fo(3, lbs=(1,), drain_scalar=True)
                dummy_mm(10)  # cover batch 0's norm-chain latency
                for fo in range(KO):
                    o_proj_group(ctxs, pr, fo, lbs=(0,), eager=True)
                # batch 1's norm chain runs under batch 0's output projection
                norm(lbs=(1,))
                dummy_mm(14)  # cover batch 1's norm-chain latency
                for fo in range(KO):
                    o_proj_group(ctxs, pr, fo, lbs=(1,), eager=True)
            else:
                norm()

            prev = (pr, ctxs)
            q16, k16, vts = q16n, k16n, vtsn

    nc.compile()
    return nc


def _get_built():
    global _BUILT
    if _BUILT is None:
        _BUILT = _build()
    return _BUILT


def _numpy_ref(query, key_in, value_in, cache, mask, Wq, bq, Wk, bk, Wv, bv, Wo, bo):
    # Fallback oracle (only used if mask/bias assumptions are violated).
    k_full = np.concatenate([cache, key_in], axis=1)
    v_full = np.concatenate([cache, value_in], axis=1)

    def proj(x, W, b):
        y = x @ W.T + b
        return y.reshape(x.shape[0], x.shape[1], H, DK).transpose(0, 2, 1, 3)

    q = proj(query, Wq, bq)
    k = proj(k_full, Wk, bk)
    v = proj(v_full, Wv, bv)
    s = np.einsum("bhqd,bhkd->bhqk", q, k) / np.sqrt(np.float32(DK))
    m = mask[:, None, :, :]
    s = np.where(m, s, -10000.0)
    s = s - s.max(-1, keepdims=True)
    e = np.exp(s)
    a = e / e.sum(-1, keepdims=True)
    a = np.where(m, a, 0.0)
    ctx = np.einsum("bhqk,bhkd->bhqd", a, v)
    ctx = ctx.transpose(0, 2, 1, 3).reshape(query.shape[0], query.shape[1], F)
    return (ctx @ Wo.T + bo).astype(np.float32)


def kernel(**inputs):
    q = np.asarray(inputs["query"], np.float32)
    key_in = np.asarray(inputs["key_in"], np.float32)
    value_in = np.asarray(inputs["value_in"], np.float32)
    cache = np.asarray(inputs["cache"], np.float32)
    mask = np.asarray(inputs["mask"])
    Wq = np.asarray(inputs["Wq"], np.float32)
    Wk = np.asarray(inputs["Wk"], np.float32)
    Wv = np.asarray(inputs["Wv"], np.float32)
    Wo = np.asarray(inputs["Wo"], np.float32)
    bq = np.asarray(inputs["bq"], np.float32)
    bk = np.asarray(inputs["bk"], np.float32)
    bv = np.asarray(inputs["bv"], np.float32)
    bo = np.asarray(inputs["bo"], np.float32)

    if (not mask.all()) or any(np.any(b != 0) for b in (bq, bk, bv, bo)):
        return _numpy_ref(q, key_in, value_in, cache, mask, Wq, bq, Wk, bk, Wv, bv, Wo, bo)

    nc = _get_built()

    def pack_w(W):
        # [fin, fout] -> [p, o, fout] with fin = o*128 + p
        return np.ascontiguousarray(
            W.T.reshape(KO, P, F).transpose(1, 0, 2)
        ).astype(np.float16)

    def pack_act(x):
        # [b, t, f] -> [pr, p, o, lb, t] with b = 2*pr+lb, f = o*128+p
        a = x.transpose(0, 2, 1).reshape(NPAIR, 2, KO, P, T1)
        return np.ascontiguousarray(a.transpose(0, 3, 2, 1, 4)).astype(np.float16)

    def pack_cache(c):
        # [b, t, f] -> [p, o, b, t]
        a = c.transpose(2, 0, 1).reshape(KO, P, NB, LC)
        return np.ascontiguousarray(a.transpose(1, 0, 2, 3)).astype(np.float16)

    wq_t, wk_t, wv_t, wo_t = pack_w(Wq), pack_w(Wk), pack_w(Wv), pack_w(Wo)

    in_maps = []
    for c in range(NCORES):
        sl = slice(c * NB, (c + 1) * NB)
        in_maps.append(
            {
                "qT": pack_act(q[sl]),
                "keyT": pack_act(key_in[sl]),
                "valT": pack_act(value_in[sl]),
                "cachT": pack_cache(cache[sl]),
                "wq": wq_t,
                "wk": wk_t,
                "wv": wv_t,
                "wo": wo_t,
            }
        )

    from concourse.bass_utils import run_bass_kernel_spmd

    res = run_bass_kernel_spmd(nc, in_maps, core_ids=list(range(NCORES)))
    kernel._last_results = res
    return np.concatenate(
        [np.ascontiguousarray(r["out"].transpose(0, 2, 1)).astype(np.float32) for r in res.results],
        axis=0,
    )



# revision 37
# speedup vs baseline: 1.0289x; 1.0289x over previous
"""Trainium2 Bass kernel for nn_MultiHeadAttention_88923002896848.

MHA with KV-cache concat: out = MHA(query; [cache;key_in]; [cache;value_in]).
Shapes: B=128, T1=188, LC=70, T2=258, F=512, H=8, DK=64. fp32 I/O.

Strategy (8 NeuronCores, data-parallel over batch, 16 batches/core):
  - Host packs all inputs into the exact on-chip layouts (partition-major),
    so every input DMA is one contiguous run per partition.
  - All matmuls fp16. Attention transposed (S^T = k-chunks x q) so exp output
    E^T feeds PV with zero on-chip transposes; exp is max-free (logits ~N(0,1))
    with a constant -3 shift; softmax denominator comes free from a ones-column
    appended to time-major V (row 64 of the PV psum).
  - Normalization runs entirely off the PE: Vector reciprocal of the
    denominator row, GpSimd partition_broadcast across the 64 head dims,
    Vector multiply into the (feature-major) context buffer.
  - Output projection is pair-folded feature-major (N=376, M=128 always);
    the kernel emits outT [b, F, T1] fp16 and the host transposes/upcasts.
  - Software pipelining across batch pairs: while pair p's attention runs,
    the PE also executes Q/K/V projections of pair p+1 and the output
    projection of pair p-1 as filler groups slotted between attention
    groups, keeping the PE dense so the HAM clock gate stays at 2.4 GHz.
"""

import numpy as np

NCORES = 8
B, T1, LC, F, H = 128, 188, 70, 512, 8
DK = F // H            # 64
T2 = LC + T1           # 258
P = 128
KO = F // P            # 4 fin/fout tiles of 128
NB = B // NCORES       # 16 batches per core
NPAIR = NB // 2
SCALE = 1.0 / np.sqrt(DK)
EXP_SHIFT = -3.0       # exp(scale*s + shift); cancels in the softmax ratio

# T2 chunks aligned to the cache/key seam: (size, (source, t0, t1))
T2_CHUNKS = [(LC, ("cache", 0, LC)), (128, ("key", 0, 128)), (T1 - 128, ("key", 128, T1))]

_BUILT = None


def _build():
    import concourse.bacc as bacc
    import concourse.mybir as mybir
    import concourse.tile as tile
    from concourse.bass import ts
    from contextlib import ExitStack

    dt = mybir.dt
    f32, f16 = dt.float32, dt.float16
    AF = mybir.ActivationFunctionType

    nc = bacc.Bacc(trn_type="TRN2")

    qT = nc.dram_tensor("qT", [NPAIR, P, KO, 2, T1], f16, kind="ExternalInput")
    keyT = nc.dram_tensor("keyT", [NPAIR, P, KO, 2, T1], f16, kind="ExternalInput")
    valT = nc.dram_tensor("valT", [NPAIR, P, KO, 2, T1], f16, kind="ExternalInput")
    cachT = nc.dram_tensor("cachT", [P, KO, NB, LC], f16, kind="ExternalInput")
    wq_d = nc.dram_tensor("wq", [P, KO, F], f16, kind="ExternalInput")
    wk_d = nc.dram_tensor("wk", [P, KO, F], f16, kind="ExternalInput")
    wv_d = nc.dram_tensor("wv", [P, KO, F], f16, kind="ExternalInput")
    wo_d = nc.dram_tensor("wo", [P, KO, F], f16, kind="ExternalInput")
    out_d = nc.dram_tensor("out", [NB, F, T1], f16, kind="ExternalOutput")

    with tile.TileContext(nc) as tc, ExitStack() as ctx:
        consts = ctx.enter_context(tc.tile_pool(name="consts", bufs=1))
        iobuf = ctx.enter_context(tc.tile_pool(name="iobuf", bufs=2))
        act16 = ctx.enter_context(tc.tile_pool(name="act16", bufs=2))
        small = ctx.enter_context(tc.tile_pool(name="small", bufs=2))
        pproj = ctx.enter_context(tc.tile_pool(name="pproj", bufs=2, space="PSUM"))
        # pss tiles are 2 banks each ([P, 2, 2, 256] f32); 2 bufs = 4 banks
        pscore = ctx.enter_context(tc.tile_pool(name="pscore", bufs=2, space="PSUM"))
        pctx = ctx.enter_context(tc.tile_pool(name="pctx", bufs=2, space="PSUM"))

        # ---- constants ----
        # DMA order: wk + cache first so the KTc preamble matmuls can start
        # as early as possible, overlapping the remaining input streams
        w_sb = {}
        for nm, drt in (("wk", wk_d), ("wq", wq_d), ("wv", wv_d), ("wo", wo_d)):
            wt = consts.tile([P, KO, F], f16, name=f"{nm}_sb", tag=f"{nm}_sb")
            w_sb[nm] = wt
        cache_all = consts.tile([P, KO, NB, LC], f16, name="cache_all")
        nc.sync.dma_start(
            w_sb["wk"][:, :2].rearrange("p o f -> p (o f)"),
            wk_d[:, :2].rearrange("p o f -> p (o f)"),
        )
        nc.sync.dma_start(
            w_sb["wk"][:, 2:].rearrange("p o f -> p (o f)"),
            wk_d[:, 2:].rearrange("p o f -> p (o f)"),
        )
        # cache lands in two halves so the first KTc chunk (batches 0-5)
        # can start while the second half streams
        nc.sync.dma_start(cache_all[:, :, :6], cachT[:, :, :6])
        nc.sync.dma_start(cache_all[:, :, 6:], cachT[:, :, 6:])
        nc.sync.dma_start(
            w_sb["wq"].rearrange("p o f -> p (o f)"),
            wq_d.rearrange("p o f -> p (o f)"),
        )
        biasm3 = consts.tile([P, 1], f32, name="biasm3")
        nc.vector.memset(biasm3[:], EXP_SHIFT)

        # ---- HAM keep-warm: dependency-free matmuls on a zero tile ----
        # The PE clock gate defaults to 4/8 (1.2 GHz) and only reaches 2.4 GHz
        # after ~3.4us of sustained activity; it re-throttles after ~3.4us
        # idle.  Dummy matmuls spin the array while real work is blocked on
        # DMA (head) or the normalization chain (tail).
        dumw = consts.tile([P, F], f16, name="dumw")
        nc.vector.memset(dumw.rearrange("p f -> p (f)"), 0.0)

        def dummy_mm(n=1):
            for _ in range(n):
                pdm = pproj.tile([P, F], f32, tag="proj", name="pdm")
                nc.tensor.matmul(
                    pdm[:], dumw[:, 0:P], dumw[:], start=True, stop=True
                )

        # spin the PE during the initial weight/cache DMA wait so the KTc
        # preamble starts already at 2.4 GHz (~8 cold matmuls = the 3.4us
        # activity window; more would overshoot the DMA wait and delay it)
        dummy_mm(8)

        # ---- zero the K-pad rows of the attention operand tiles ----
        # PV matmuls run with a full K=128 contraction (pad rows contribute
        # exact zeros) so the weight-load path sees 128-row stationaries and
        # overlaps LDWEIGHTS with the running matmul, like the projections.
        # Both rotation buffers of each tag are zeroed once; steady-state
        # writers only ever touch rows [:tcn], so the pads stay zero.
        for _ in range(2):
            for ci, (tcn, _) in enumerate(T2_CHUNKS):
                if tcn < P:
                    Ez = act16.tile(
                        [P, KO, 2, 2, T1], f16, tag=f"E{ci}", name="Ez", bufs=2
                    )
                    nc.vector.memset(
                        Ez.rearrange("p a b c t -> p (a b c t)"), 0.0
                    )
                    for lb in range(2):
                        vz = act16.tile(
                            [P, H, DK + 1], f16, tag=f"vt{lb}{ci}", name="vz", bufs=2
                        )
                        nc.vector.memset(
                            vz.rearrange("p h d -> p (h d)"), 0.0
                        )

        # ---- K projection of all cache frames (feature-major, fp16) ----
        # chunk-outer so chunk 0 (cache batches 0-5) only waits for the
        # first half of the cache DMA
        KTc = consts.tile([P, KO, NB, LC], f16, name="KTc")
        NTOT = NB * LC  # 1120
        cch = [(0, 374), (374, 374), (748, NTOT - 748)]
        for c0, cn in cch:
            for fo in range(KO):
                pkc = pproj.tile([P, F], f32, tag="proj", name="pkc")
                for k in range(KO):
                    nc.tensor.matmul(
                        pkc[:, :cn],
                        w_sb["wk"][:, k, ts(fo, P)],
                        cache_all[:, k].rearrange("p b t -> p (b t)")[:, c0 : c0 + cn],
                        start=(k == 0),
                        stop=(k == KO - 1),
                    )
                nc.scalar.copy(
                    KTc[:, fo].rearrange("p b t -> p (b t)")[:, c0 : c0 + cn],
                    pkc[:, :cn],
                )

        # ================= pipelined stages =================

        def dma_pair(pr):
            qp = iobuf.tile([P, KO, 2, T1], f16, tag="qp", name="qp")
            kp = iobuf.tile([P, KO, 2, T1], f16, tag="kp", name="kp")
            vp = iobuf.tile([P, KO, 2, T1], f16, tag="vp", name="vp")
            nc.sync.dma_start(
                qp.rearrange("p o b t -> p (o b t)"),
                qT[pr].rearrange("p o b t -> p (o b t)"),
            )
            nc.sync.dma_start(
                kp.rearrange("p o b t -> p (o b t)"),
                keyT[pr].rearrange("p o b t -> p (o b t)"),
            )
            nc.sync.dma_start(
                vp.rearrange("p o b t -> p (o b t)"),
                valT[pr].rearrange("p o b t -> p (o b t)"),
            )
            return qp, kp, vp

        def qk_proj_group(io, w, dst, fo):
            # one fo block of the pair-folded Q or K projection (N=376)
            pq = pproj.tile([P, F], f32, tag="proj", name="pq")
            for k in range(KO):
                nc.tensor.matmul(
                    pq[:, : 2 * T1],
                    w_sb[w][:, k, ts(fo, P)],
                    io[:, k].rearrange("p b t -> p (b t)"),
                    start=(k == 0),
                    stop=(k == KO - 1),
                )
            if w == "wq":
                nc.scalar.copy(dst[:, fo].rearrange("p b t -> p (b t)"), pq[:, : 2 * T1])
            else:
                nc.vector.tensor_copy(
                    dst[:, fo].rearrange("p b t -> p (b t)"), pq[:, : 2 * T1]
                )

        def v_proj_group(vp, pr, lb, ci, vts):
            # one T2 chunk of batch (pr, lb)'s V projection, time-major + ones col
            tcn, (src, s0, s1) = T2_CHUNKS[ci]
            b = 2 * pr + lb
            pv = pproj.tile([P, F], f32, tag="proj", name="pv")[:tcn]
            for k in range(KO):
                if src == "cache":
                    lhsT = cache_all[:, k, b, :]
                else:
                    lhsT = vp[:, k, lb, s0:s1]
                nc.tensor.matmul(
                    pv[:, :],
                    lhsT,
                    w_sb["wv"][:, k],
                    start=(k == 0),
                    stop=(k == KO - 1),
                )
            vt = vts[lb][ci]
            nc.vector.tensor_copy(vt[:tcn, :, 0:DK], pv.rearrange("t (h d) -> t h d", d=DK))
            nc.gpsimd.memset(vt[:tcn, :, DK : DK + 1], 1.0)

        obp_live = {}

        def o_proj_group(ctxs, pr, fo, lbs=(0, 1), eager=False):
            # one fo block of the pair-folded output projection (N=376),
            # feature-major output; lbs selects batches (epilogue splits them).
            # Drains accumulate in a per-(pair, lbs) staging tile; the DRAM
            # store is one batched DMA per batch after the last fo block.
            nlb = len(lbs)
            po = pproj.tile([P, F], f32, tag="proj", name="po")
            for k in range(KO):
                if nlb == 2:
                    rhs = ctxs[:, k].rearrange("p b t -> p (b t)")
                else:
                    rhs = ctxs[:, k, lbs[0], :]
                nc.tensor.matmul(
                    po[:, : nlb * T1],
                    w_sb["wo"][:, k, ts(fo, P)],
                    rhs,
                    start=(k == 0),
                    stop=(k == KO - 1),
                )
            if eager:
                # epilogue: store each fo block as soon as it drains (shorter
                # critical path than the batched whole-row store), and drain
                # on DVE since the scalar engine is saturated with exp here
                ob = small.tile([P, 2, T1], f16, tag="obe", name="obe", bufs=3)
                nc.vector.tensor_copy(
                    ob[:, :nlb].rearrange("p b t -> p (b t)"), po[:, : nlb * T1]
                )
                for i, lb in enumerate(lbs):
                    nc.sync.dma_start(out_d[2 * pr + lb, ts(fo, P), :], ob[:, i, :])
                return
            key = (pr, lbs)
            if key not in obp_live:
                obp_live[key] = small.tile(
                    [P, KO, 2, T1], f16, tag="obp", name="obp", bufs=2
                )
            obp = obp_live[key]
            nc.scalar.copy(
                obp[:, fo, :nlb].rearrange("p b t -> p (b t)"), po[:, : nlb * T1]
            )
            if fo == KO - 1:
                for i, lb in enumerate(lbs):
                    b = 2 * pr + lb
                    nc.sync.dma_start(
                        out_d[b].rearrange("(o p) t -> p o t", p=P),
                        obp[:, :, i, :],
                    )
                del obp_live[key]

        # ---- prologue: pair 0 inputs + projections ----
        qp0, kp0, vp0 = dma_pair(0)
        for nm, drt in (("wv", wv_d), ("wo", wo_d)):
            nc.sync.dma_start(
                w_sb[nm].rearrange("p o f -> p (o f)"),
                drt.rearrange("p o f -> p (o f)"),
            )
        q16 = act16.tile([P, KO, 2, T1], f16, tag="q16", name="q16")
        k16 = act16.tile([P, KO, 2, T1], f16, tag="k16", name="k16")
        for fo in range(KO):
            qk_proj_group(qp0, "wq", q16, fo)
        for fo in range(KO):
            qk_proj_group(kp0, "wk", k16, fo)
        vts = [
            [
                act16.tile([P, H, DK + 1], f16, tag=f"vt{lb}{ci}", name=f"vt{lb}{ci}", bufs=2)
                for ci, (tcn, _) in enumerate(T2_CHUNKS)
            ]
            for lb in range(2)
        ]
        for lb in range(2):
            for ci in range(len(T2_CHUNKS)):
                v_proj_group(vp0, 0, lb, ci, vts)

        prev = None  # (pr, ctxs) of previous pair, for the deferred O projection
        vts_deferred = None  # (vp, vts) of the last pair, projected in-pair

        for pr in range(NPAIR):
            # queue next pair's DMA + build its filler group list.  The last
            # pair's V projection is deferred into the last pair itself (VP
            # fillers first there) so the tail keeps the PE dense.
            fillers = []
            if pr == NPAIR - 1 and vts_deferred is not None:
                vpn_d, vtsn_d = vts_deferred
                for lb in range(2):
                    for ci in range(len(T2_CHUNKS)):
                        fillers.append(
                            lambda lb=lb, ci=ci: v_proj_group(vpn_d, pr, lb, ci, vtsn_d)
                        )
                vts = vtsn_d
            if pr + 1 < NPAIR:
                qpn, kpn, vpn = dma_pair(pr + 1)
                q16n = act16.tile([P, KO, 2, T1], f16, tag="q16", name="q16n")
                k16n = act16.tile([P, KO, 2, T1], f16, tag="k16", name="k16n")
                vtsn = [
                    [
                        act16.tile(
                            [P, H, DK + 1], f16, tag=f"vt{lb}{ci}", name=f"vtn{lb}{ci}", bufs=2
                        )
                        for ci, (tcn, _) in enumerate(T2_CHUNKS)
                    ]
                    for lb in range(2)
                ]
                for fo in range(KO):
                    fillers.append(lambda fo=fo: qk_proj_group(qpn, "wq", q16n, fo))
                for fo in range(KO):
                    fillers.append(lambda fo=fo: qk_proj_group(kpn, "wk", k16n, fo))
                if pr + 1 == NPAIR - 1:
                    vts_deferred = (vpn, vtsn)
                    vtsn = None
                else:
                    for lb in range(2):
                        for ci in range(len(T2_CHUNKS)):
                            fillers.append(
                                lambda lb=lb, ci=ci: v_proj_group(vpn, pr + 1, lb, ci, vtsn)
                            )
            else:
                q16n = k16n = vtsn = None
            if prev is not None:
                ppr, pctxs = prev
                for fo in range(KO):
                    fillers.append(lambda fo=fo: o_proj_group(pctxs, ppr, fo))
            fillers.reverse()  # pop() from the front

            def filler(n=1):
                for _ in range(n):
                    if fillers:
                        fillers.pop()()
                    elif pr == NPAIR - 1:
                        # no real work left: keep the PE clock gate warm
                        dummy_mm()

            q16c, k16c, vtsc = q16, k16, vts

            # E tiles for the pair: [tcn, fo, j, lb, t] fp16 (h = 2*fo + j)
            E = [
                act16.tile([P, KO, 2, 2, T1], f16, tag=f"E{ci}", name=f"E{ci}", bufs=2)
                for ci, (tcn, _) in enumerate(T2_CHUNKS)
            ]
            cu = [
                small.tile([DK + 1, H, T1], f16, tag=f"cu{lb}", name=f"cu{lb}", bufs=2)
                for lb in range(2)
            ]
            ctxs = small.tile([P, KO, 2, T1], f16, tag="ctxs", name="ctxs", bufs=2)

            def scores_fo(fo):
                # S^T + exp for head pair fo, both batches, all chunks.
                # One 2-bank psum tile per chunk holds all four (j, lb) slots;
                # emission alternates j so consecutive matmuls land on
                # disjoint PE row halves (rows 0-63 vs 64-127) and run
                # concurrently via tile_position row groups.  lb=0 starts
                # (whole-bank clear), lb=1 lands on cleared has_written bits.
                for ci, (tcn, (src, s0, s1)) in enumerate(T2_CHUNKS):
                    pss = pscore.tile([P, 2, 2, 256], f32, tag="pss", name="pss")[:tcn]
                    for lb in range(2):
                        for j in range(2):
                            if src == "cache":
                                lhsT = KTc[ts(j, DK), fo, 2 * pr + lb, :]
                            else:
                                lhsT = k16c[ts(j, DK), fo, lb, s0:s1]
                            nc.tensor.matmul(
                                pss[:, j, lb, :T1],
                                lhsT,
                                q16c[ts(j, DK), fo, lb, :],
                                start=(lb == 0),
                                stop=(lb == 1),
                                skip_group_check=True,
                            )
                    # one wide activation covers both heads and both batches
                    nc.scalar.activation(
                        E[ci][:tcn, fo, :, :, :],
                        pss[:, :, :, :T1],
                        AF.Exp,
                        bias=biasm3[:tcn, :],
                        scale=SCALE,
                    )

            def pv_fo(fo, lbs=(0, 1), drain_scalar=False):
                # PV with fused denominator row; drain unnormalized to cu (fp16)
                for lb in lbs:
                    pc = pctx.tile([DK + 1, 2, 256], f32, tag="pc", name="pc")
                    for j in range(2):
                        h = 2 * fo + j
                        for ci, (tcn, _) in enumerate(T2_CHUNKS):
                            # both heads share one accumulation group (j=1's
                            # first write overwrites on cleared bits) to avoid
                            # a second bank-clear and mid-group drain tail.
                            # K is padded to the full 128 rows (pads are zero)
                            # so the weight load takes the fast 128-row path.
                            nc.tensor.matmul(
                                pc[:, j, :T1],
                                vtsc[lb][ci][:, h, :],
                                E[ci][:, fo, j, lb, :],
                                start=(j == 0 and ci == 0),
                                stop=(j == 1 and ci == len(T2_CHUNKS) - 1),
                            )
                    if drain_scalar:
                        # epilogue: scalar is free after the last exp, and
                        # keeping these drains off the DVE lets the norm
                        # chains start the moment their cu rows complete
                        nc.scalar.copy(
                            cu[lb][:, 2 * fo : 2 * fo + 2, :], pc[:, :, :T1]
                        )
                    else:
                        nc.vector.tensor_copy(
                            cu[lb][:, 2 * fo : 2 * fo + 2, :], pc[:, :, :T1]
                        )

            # ---- attention sequence with interleaved filler groups ----
            if pr < NPAIR - 1:
                scores_fo(0)
                filler(2)
                scores_fo(1)
                filler(2)
                pv_fo(0)
                filler(2)
                scores_fo(2)
                filler(2)
                pv_fo(1)
                filler(2)
                scores_fo(3)
                filler(2)
                pv_fo(2)
                filler(2)
                pv_fo(3)
                while fillers:
                    filler(1)

            # ---- normalization (no PE): recip + partition broadcast + mul ----
            # The two batches' denominator rows (2x1504 fp16 on one partition)
            # are DMA-packed onto 94 partitions so the f32 cast + reciprocal +
            # f16 cast run at FD=32 instead of FD=1504 per instruction, then
            # DMA-unpacked back to single-partition layout for the gpsimd
            # broadcast.  Saves ~7us/pair of single-lane DVE work.
            def norm(lbs=(0, 1)):
                nlb = len(lbs)
                dpk = small.tile([94, 32], f16, tag="dpk", name="dpk", bufs=2)
                for i, lb in enumerate(lbs):
                    nc.sync.dma_start(
                        dpk[47 * i : 47 * (i + 1), :],
                        cu[lb][DK : DK + 1, :, :].rearrange("p h t -> p (h t)"),
                    )
                dpk32 = small.tile([94, 32], f32, tag="dpk32", name="dpk32", bufs=2)
                nc.vector.tensor_copy(dpk32[: 47 * nlb], dpk[: 47 * nlb])
                rpk32 = small.tile([94, 32], f32, tag="rpk32", name="rpk32", bufs=2)
                nc.vector.reciprocal_approx_fast(
                    out=rpk32[: 47 * nlb], in_=dpk32[: 47 * nlb]
                )
                rpk = small.tile([94, 32], f16, tag="rpk", name="rpk", bufs=2)
                nc.vector.tensor_copy(rpk[: 47 * nlb], rpk32[: 47 * nlb])
                rjs = {}
                for i, lb in enumerate(lbs):
                    rj = small.tile([1, H, T1], f16, tag=f"rj{lb}", name=f"rj{lb}", bufs=2)
                    nc.sync.dma_start(
                        rj.rearrange("p h t -> p (h t)"),
                        rpk[47 * i : 47 * (i + 1), :],
                    )
                    rjs[lb] = rj
                for lb in lbs:
                    for j in range(2):
                        rb = small.tile(
                            [DK, KO, T1], f16, tag=f"rb{j}", name=f"rb{j}", bufs=2
                        )
                        nc.gpsimd.partition_broadcast(
                            rb[:],
                            rjs[lb].rearrange("p (f j) t -> p j f t", j=2)[:, j],
                        )
                        nc.vector.tensor_mul(
                            ctxs[ts(j, DK), :, lb, :],
                            cu[lb][0:DK, :, :].rearrange("p (f j) t -> p j f t", j=2)[
                                :, j
                            ],
                            rb[:],
                        )

            if pr == NPAIR - 1:
                # final pair: drain batch 0's attention first so its
                # normalization and output projection overlap batch 1's PV,
                # keeping the PE busy through the pipeline tail
                scores_fo(0)
                filler(3)
                scores_fo(1)
                filler(3)
                pv_fo(0, lbs=(0,))
                scores_fo(2)
                filler(2)
                pv_fo(1, lbs=(0,))
                scores_fo(3)
                filler(2)
                pv_fo(2, lbs=(0,))
                filler(1)
                pv_fo(3, lbs=(0,))
                while fillers:
                    filler(1)
                # batch 1's cu drains go to scalar (its exp work is done) so
                # batch 1's norm chain isn't queued behind DVE work
                pv_fo(0, lbs=(1,), drain_scalar=True)
                pv_fo(1, lbs=(1,), drain_scalar=True)
                # batch 0's norm chain (DMA/DVE/gpsimd) runs under batch 1's PV
                norm(lbs=(0,))
                pv_fo(2, lbs=(1,), drain_scalar=True)
                pv_fo(3, lbs=(1,), drain_scalar=True)
                dummy_mm(10)  # cover batch 0's norm-chain latency
                for fo in range(KO):
                    o_proj_group(ctxs, pr, fo, lbs=(0,), eager=True)
                # batch 1's norm chain runs under batch 0's output projection
                norm(lbs=(1,))
                dummy_mm(14)  # cover batch 1's norm-chain latency
                for fo in range(KO):
                    o_proj_group(ctxs, pr, fo, lbs=(1,), eager=True)
            else:
                norm()

            prev = (pr, ctxs)
            q16, k16, vts = q16n, k16n, vtsn

    nc.compile()
    return nc


def _get_built():
    global _BUILT
    if _BUILT is None:
        _BUILT = _build()
    return _BUILT


def _numpy_ref(query, key_in, value_in, cache, mask, Wq, bq, Wk, bk, Wv, bv, Wo, bo):
    # Fallback oracle (only used if mask/bias assumptions are violated).
    k_full = np.concatenate([cache, key_in], axis=1)
    v_full = np.concatenate([cache, value_in], axis=1)

    def proj(x, W, b):
        y = x @ W.T + b
        return y.reshape(x.shape[0], x.shape[1], H, DK).transpose(0, 2, 1, 3)

    q = proj(query, Wq, bq)
    k = proj(k_full, Wk, bk)
    v = proj(v_full, Wv, bv)
    s = np.einsum("bhqd,bhkd->bhqk", q, k) / np.sqrt(np.float32(DK))
    m = mask[:, None, :, :]
    s = np.where(m, s, -10000.0)
    s = s - s.max(-1, keepdims=True)
    e = np.exp(s)
    a = e / e.sum(-1, keepdims=True)
    a = np.where(m, a, 0.0)
    ctx = np.einsum("bhqk,bhkd->bhqd", a, v)
    ctx = ctx.transpose(0, 2, 1, 3).reshape(query.shape[0], query.shape[1], F)
    return (ctx @ Wo.T + bo).astype(np.float32)


def kernel(**inputs):
    q = np.asarray(inputs["query"], np.float32)
    key_in = np.asarray(inputs["key_in"], np.float32)
    value_in = np.asarray(inputs["value_in"], np.float32)
    cache = np.asarray(inputs["cache"], np.float32)
    mask = np.asarray(inputs["mask"])
    Wq = np.asarray(inputs["Wq"], np.float32)
    Wk = np.asarray(inputs["Wk"], np.float32)
    Wv = np.asarray(inputs["Wv"], np.float32)
    Wo = np.asarray(inputs["Wo"], np.float32)
    bq = np.asarray(inputs["bq"], np.float32)
    bk = np.asarray(inputs["bk"], np.float32)
    bv = np.asarray(inputs["bv"], np.float32)
    bo = np.asarray(inputs["bo"], np.float32)

    if (not mask.all()) or any(np.any(b != 0) for b in (bq, bk, bv, bo)):
        return _numpy_ref(q, key_in, value_in, cache, mask, Wq, bq, Wk, bk, Wv, bv, Wo, bo)

    nc = _get_built()

    def pack_w(W):
        # [fin, fout] -> [p, o, fout] with fin = o*128 + p
        return np.ascontiguousarray(
            W.T.reshape(KO, P, F).transpose(1, 0, 2)
        ).astype(np.float16)

    def pack_act(x):
        # [b, t, f] -> [pr, p, o, lb, t] with b = 2*pr+lb, f = o*128+p
        a = x.transpose(0, 2, 1).reshape(NPAIR, 2, KO, P, T1)
        return np.ascontiguousarray(a.transpose(0, 3, 2, 1, 4)).astype(np.float16)

    def pack_cache(c):
        # [b, t, f] -> [p, o, b, t]
        a = c.transpose(2, 0, 1).reshape(KO, P, NB, LC)
        return np.ascontiguousarray(a.transpose(1, 0, 2, 3)).astype(np.float16)

    wq_t, wk_t, wv_t, wo_t = pack_w(Wq), pack_w(Wk), pack_w(Wv), pack_w(Wo)

    in_maps = []
    for c in range(NCORES):
        sl = slice(c * NB, (c + 1) * NB)
        in_maps.append(
            {
                "qT": pack_act(q[sl]),
                "keyT": pack_act(key_in[sl]),
                "valT": pack_act(value_in[sl]),
                "cachT": pack_cache(cache[sl]),
                "wq": wq_t,
                "wk": wk_t,
                "wv": wv_t,
                "wo": wo_t,
            }
        )

    from concourse.bass_utils import run_bass_kernel_spmd

    res = run_bass_kernel_spmd(nc, in_maps, core_ids=list(range(NCORES)))
    kernel._last_results = res
    return np.concatenate(
        [np.ascontiguousarray(r["out"].transpose(0, 2, 1)).astype(np.float32) for r in res.results],
        axis=0,
    )

